# revision 1
# baseline (speedup 1.0000x reference)
"""Causal MHA forward on 8 NeuronCores (Trainium2, Bass/Tile).

Sharding: batch (4) x head-half (2) -> 8 cores. Each core computes, for its
batch b and its 8 heads: QKV column-sliced projections, causal attention in
transposed-score layout (S^T[k, q] so softmax rowsums come from a
ones-augmented V matmul and no transposes are needed), and a partial dense
projection against the matching 512-row slice of dense_w. The host sums the
two partial dense outputs per batch and adds dense_b + wv_b @ dense_w
(valid because softmax rows sum to 1).
"""
import numpy as np

import concourse.bacc as bacc
import concourse.bass as bass
import concourse.tile as tile
import concourse.mybir as mybir
from concourse.bass_utils import run_bass_kernel_spmd

B, S, D, = 4, 2048, 1024
DC = 512           # per-core d slice (8 heads x 64)
H = 8              # heads per core
DH = 64
N_CORES = 8
F32 = mybir.dt.float32
AF = mybir.ActivationFunctionType
NEG = -1.0e9
SCALE = 1.0 / 32.0  # 1/sqrt(D_MODEL)
F32R = mybir.dt.float32r


def _r(ap):
    return ap


_CACHE = {}


def _build():
    nc = bacc.Bacc("TRN2", target_bir_lowering=False, debug=False,
                   num_devices=N_CORES)
    xt = nc.dram_tensor("xt", [D, S], F32R, kind="ExternalInput")
    wq = nc.dram_tensor("wq", [D, DC], F32R, kind="ExternalInput")
    wk = nc.dram_tensor("wk", [D, DC], F32R, kind="ExternalInput")
    wv = nc.dram_tensor("wv", [D, DC], F32R, kind="ExternalInput")
    qb = nc.dram_tensor("qb", [DC], F32, kind="ExternalInput")
    kb = nc.dram_tensor("kb", [DC], F32, kind="ExternalInput")
    wd = nc.dram_tensor("wd", [DC, D], F32R, kind="ExternalInput")
    band = nc.dram_tensor("band", [128, 2048], F32R, kind="ExternalInput")
    ident = nc.dram_tensor("ident", [128, 128], F32R, kind="ExternalInput")
    ones = nc.dram_tensor("ones", [128, 64], F32R, kind="ExternalInput")
    out = nc.dram_tensor("out", [S, D], F32, kind="ExternalOutput")

    with tile.TileContext(nc) as tc:
      with nc.allow_low_precision(reason="float32r is 4-byte storage; psum accum stays fp32"):
        with (
            tc.tile_pool(name="consts", bufs=1) as consts,
            tc.tile_pool(name="wqp", bufs=1) as wqp,
            tc.tile_pool(name="ktp", bufs=1) as ktp,
            tc.tile_pool(name="vap", bufs=1) as vap,
            tc.tile_pool(name="otp", bufs=1) as otp,
            tc.tile_pool(name="xts", bufs=3) as xtsp,
            tc.tile_pool(name="qtp", bufs=2) as qtp,
            tc.tile_pool(name="ptp", bufs=2) as ptp,
            tc.tile_pool(name="nrm", bufs=1) as nrm,
            tc.tile_pool(name="psb", bufs=2, space="PSUM") as psb,
            tc.tile_pool(name="psv", bufs=1, space="PSUM") as psv,
            tc.tile_pool(name="psm", bufs=1, space="PSUM") as psm,
        ):
            band_sb = consts.tile([128, 2048], F32R)
            nc.scalar.dma_start(out=band_sb, in_=band[:, :])
            id_sb = consts.tile([128, 128], F32R)
            nc.scalar.dma_start(out=id_sb, in_=ident[:, :])
            on_sb = consts.tile([128, 64], F32R)
            nc.scalar.dma_start(out=on_sb, in_=ones[:, :])
            qb_sb = consts.tile([128, 4], F32)
            nc.scalar.dma_start(out=qb_sb, in_=qb.ap().rearrange("(c p) -> p c", p=128))
            kb_sb = consts.tile([128, 4], F32)
            nc.scalar.dma_start(out=kb_sb, in_=kb.ap().rearrange("(c p) -> p c", p=128))

            wq_sb = wqp.tile([128, 8, DC], F32R)
            nc.scalar.dma_start(out=wq_sb, in_=wq.ap().rearrange("(c p) d -> p c d", p=128))

            kt = ktp.tile([128, 4, S], F32R)       # K^T, pair p rows = d 128p..
            va = vap.tile([128, 16, H, 65], F32R)  # V + ones col, per s-block
            ot = otp.tile([128, 4, S], F32R)       # O^T accumulated
            nc.vector.memset(va[:, :, :, 64:65].bitcast(F32), 1.0)

            # ---- Phase 1: K^T and V projections (stream x^T by s-chunk) ----
            with tc.tile_pool(name="wkv", bufs=1) as wkvp:
                wk_sb = wkvp.tile([128, 8, DC], F32R)
                wv_sb = wkvp.tile([128, 8, DC], F32R)

                def load_wk():
                    nc.sync.dma_start(out=wk_sb, in_=wk.ap().rearrange("(c p) d -> p c d", p=128))
                qts = {}

                def qproj(cc, xg_):
                    qt_ = qtp.tile([128, 4, 512], F32R, tag="qt", name=f"qt{cc}")
                    for p in range(4):
                        ps = psm.tile([128, 512], F32, tag="mm", name="qproj")
                        for i in range(8):
                            nc.tensor.matmul(ps, _r(wq_sb[:, i, 128 * p:128 * (p + 1)]),
                                             _r(xg_[i // 4][:, i % 4, :]),
                                             start=(i == 0), stop=(i == 7))
                        nc.vector.tensor_scalar_add(out=qt_[:, p, :], in0=ps,
                                                    scalar1=qb_sb[:, p:p + 1])
                    qts[cc] = qt_

                def load_xts(cc, order=None):
                    a = xtsp.tile([128, 4, 512], F32R, tag="xts", name="xts0")
                    b = xtsp.tile([128, 4, 512], F32R, tag="xts", name="xts1")
                    xv = xt.ap().rearrange("(i p) s -> p i s", p=128)
                    da = lambda: nc.sync.dma_start(
                        out=a, in_=xv[:, 0:4, 512 * cc:512 * (cc + 1)])
                    db = lambda: nc.sync.dma_start(
                        out=b, in_=xv[:, 4:8, 512 * cc:512 * (cc + 1)])
                    if order is None:
                        da(); db()
                    else:
                        da(); order(); db()
                    return [a, b]

                for sc in range(4):
                    if sc == 0:
                        xg = load_xts(sc, order=load_wk)
                        nc.sync.dma_start(out=wv_sb, in_=wv.ap().rearrange("(c p) d -> p c d", p=128))
                    else:
                        xg = load_xts(sc)
                    for p in range(4):
                        ps = psv.tile([128, 512], F32, tag="pvA", bufs=2, name="kvps")
                        for i in range(8):
                            nc.tensor.matmul(ps, _r(wk_sb[:, i, 128 * p:128 * (p + 1)]),
                                             _r(xg[i // 4][:, i % 4, :]),
                                             start=(i == 0), stop=(i == 7))
                        nc.vector.tensor_scalar_add(
                            out=kt[:, p, 512 * sc:512 * (sc + 1)], in0=ps,
                            scalar1=kb_sb[:, p:p + 1])
                    for sb_ in range(4):
                        ps = psv.tile([128, 512], F32, tag="pvA", bufs=2, name="kvps")
                        for i in range(8):
                            nc.tensor.matmul(ps, _r(xg[i // 4][:, i % 4, 128 * sb_:128 * (sb_ + 1)]),
                                             _r(wv_sb[:, i, :]), start=(i == 0), stop=(i == 7))
                        sblk = 4 * sc + sb_
                        nc.vector.tensor_copy(
                            out=va[:, sblk, :, 0:64],
                            in_=ps.rearrange("p (h d) -> p h d", h=H))
                    if sc == 0:
                        qproj(0, xg)

            # ---- Phase 2+3: attention + dense, chunk at a time ----
            with (
                tc.tile_pool(name="wdp", bufs=1) as wdp,
                tc.tile_pool(name="outp", bufs=3) as outp,
            ):
                wd_sb = wdp.tile([128, 4, D], F32R)
                nc.scalar.dma_start(out=wd_sb, in_=wd.ap().rearrange("(c p) d -> p c d", p=128))
                for c in range(4):
                    if c < 3:
                        qproj(c + 1, load_xts(c + 1))
                    qt = qts[c]
                    nj = 4 * c + 4
                    for p in range(4):
                        pvA = psv.tile([65, 512], F32, tag="pvA", bufs=2, name="pvA")
                        pvB = psv.tile([65, 512], F32, tag="pvB", bufs=1, name="pvB")
                        for j in range(nj):
                            sc_ps = psb.tile([128, 1024], F32)
                            is_band = j >= 4 * c
                            nc.tensor.matmul(sc_ps[:, 0:512],
                                             _r(kt[0:64, p, 128 * j:128 * (j + 1)]),
                                             _r(qt[0:64, p, :]),
                                             start=True, stop=not is_band)
                            nc.tensor.matmul(sc_ps[:, 512:1024],
                                             _r(kt[64:128, p, 128 * j:128 * (j + 1)]),
                                             _r(qt[64:128, p, :]),
                                             start=True, stop=not is_band)
                            if is_band:
                                jj = j - 4 * c
                                m = band_sb[:, 512 * jj:512 * (jj + 1)]
                                nc.tensor.matmul(sc_ps[:, 0:512], _r(id_sb), _r(m),
                                                 start=False, stop=True)
                                nc.tensor.matmul(sc_ps[:, 512:1024], _r(id_sb), _r(m),
                                                 start=False, stop=True)
                            pt = ptp.tile([128, 1024], F32R)
                            nc.scalar.activation(out=pt, in_=sc_ps, func=AF.Exp,
                                                 scale=SCALE)
                            nc.tensor.matmul(pvA, _r(va[:, j, 2 * p, :]), _r(pt[:, 0:512]),
                                             start=(j == 0), stop=(j == nj - 1))
                            nc.tensor.matmul(pvB, _r(va[:, j, 2 * p + 1, :]), _r(pt[:, 512:1024]),
                                             start=(j == 0), stop=(j == nj - 1))
                        rr = nrm.tile([128, 1024], F32R, tag="rr")
                        nc.vector.reciprocal(out=rr[64:65, 0:512], in_=pvA[64:65, :])
                        nc.vector.reciprocal(out=rr[64:65, 512:1024], in_=pvB[64:65, :])
                        bcA = psm.tile([128, 512], F32, tag="mm", name="bcA")
                        nc.tensor.matmul(bcA[0:64, :], _r(on_sb[64:65, :]), _r(rr[64:65, 0:512]),
                                         start=True, stop=True, tile_position=(64, 0))
                        nc.vector.tensor_copy(out=rr[0:64, 0:512], in_=bcA[0:64, :])
                        nc.vector.tensor_mul(out=ot[0:64, p, 512 * c:512 * (c + 1)],
                                             in0=pvA[0:64, :], in1=rr[0:64, 0:512])
                        nc.vector.tensor_copy(out=rr[0:64, 512:1024], in_=pvB[0:64, :])
                        bcB = psm.tile([128, 512], F32, tag="mm", name="bcB")
                        nc.tensor.matmul(bcB[0:64, :], _r(on_sb[64:65, :]), _r(rr[64:65, 512:1024]),
                                         start=True, stop=True, tile_position=(64, 0))
                        nc.vector.tensor_mul(out=rr[0:64, 512:1024], in0=rr[0:64, 512:1024],
                                             in1=bcB[0:64, :])
                        sh = psm.tile([128, 512], F32, tag="mm", name="sh")
                        nc.tensor.matmul(sh[64:128, :], id_sb[0:64, 0:64].bitcast(F32), rr[0:64, 512:1024].bitcast(F32),
                                         start=True, stop=True, tile_position=(0, 64))
                        nc.vector.tensor_copy(out=ot[64:128, p, 512 * c:512 * (c + 1)],
                                              in_=sh[64:128, :])
                    # dense for this chunk's 4 s-blocks
                    for sb_ in range(4 * c, 4 * c + 4):
                        os = outp.tile([128, 1024], F32)
                        for n in range(2):
                            ps = psv.tile([128, 512], F32, tag="pvA", bufs=2, name="dps")
                            for p in range(4):
                                nc.tensor.matmul(ps, _r(ot[:, p, 128 * sb_:128 * (sb_ + 1)]),
                                                 _r(wd_sb[:, p, 512 * n:512 * (n + 1)]),
                                                 start=(p == 0), stop=(p == 3))
                            nc.vector.tensor_copy(out=os[:, 512 * n:512 * (n + 1)], in_=ps)
                        nc.sync.dma_start(out=out[128 * sb_:128 * (sb_ + 1), :], in_=os)
    nc.compile()
    return nc


def get_nc():
    if "nc" not in _CACHE:
        _CACHE["nc"] = _build()
    return _CACHE["nc"]


def kernel(x, mask, wq_w, wq_b, wk_w, wk_b, wv_w, wv_b, dense_w, dense_b,
           _trace=False):
    x = np.asarray(x, dtype=np.float32)
    wq_w = np.asarray(wq_w, np.float32); wq_b = np.asarray(wq_b, np.float32)
    wk_w = np.asarray(wk_w, np.float32); wk_b = np.asarray(wk_b, np.float32)
    wv_w = np.asarray(wv_w, np.float32); wv_b = np.asarray(wv_b, np.float32)
    dense_w = np.asarray(dense_w, np.float32)
    dense_b = np.asarray(dense_b, np.float32)

    # causal band masks M_jj[k, q'] = -1e9 where q' < 128*jj + k, cols jj*512..
    band = np.zeros((128, 2048), np.float32)
    k_idx = np.arange(128)[:, None]
    q_idx = np.arange(512)[None, :]
    for jj in range(4):
        band[:, 512 * jj:512 * (jj + 1)] = np.where(q_idx < 128 * jj + k_idx, NEG, 0.0)
    ident = np.eye(128, dtype=np.float32)
    ones = np.ones((128, 64), np.float32)

    in_maps = []
    for core in range(N_CORES):
        b, hh = divmod(core, 2)
        sl = slice(DC * hh, DC * (hh + 1))
        in_maps.append({
            "xt": np.ascontiguousarray(x[b].T),
            "wq": np.ascontiguousarray(wq_w[:, sl]),
            "wk": np.ascontiguousarray(wk_w[:, sl]),
            "wv": np.ascontiguousarray(wv_w[:, sl]),
            "qb": np.ascontiguousarray(wq_b[sl]),
            "kb": np.ascontiguousarray(wk_b[sl]),
            "wd": np.ascontiguousarray(dense_w[sl, :]),
            "band": band, "ident": ident, "ones": ones,
        })
    nc = get_nc()
    res = run_bass_kernel_spmd(nc, in_maps, core_ids=list(range(N_CORES)),
                               trace=_trace)
    const = dense_b + wv_b @ dense_w  # bias terms deferred to host
    outs = np.empty((B, S, D), np.float32)
    for b in range(B):
        outs[b] = res.results[2 * b]["out"] + res.results[2 * b + 1]["out"] + const
    if _trace:
        kernel.last_result = res
    return outs



# revision 3
# speedup vs baseline: 1.1665x; 1.1665x over previous
"""Causal MHA forward on 8 NeuronCores (Trainium2, Bass/Tile).

Sharding: batch (4) x head-half (2) -> 8 cores. Each core computes, for its
batch b and its 8 heads: QKV column-sliced projections (bf16), causal
attention in transposed-score layout (S^T[k, q]), and a partial dense
projection against the matching 512-row slice of dense_w. The host sums the
two partial dense outputs per batch and adds dense_b + wv_b @ dense_w
(valid because softmax rows sum to 1).

Key layout tricks:
- Scores/PV computed per head-pair p (heads 2p, 2p+1). Head 2p's PV lhsT is
  [V_A | ones] -> psum rows 0:64 = O_A, row 64 = rowsum_A. Head 2p+1's lhsT
  is [ones | zeros*63 | V_B] -> psum row 0 = rowsum_B, rows 64:128 = O_B, so
  both heads' outputs land at their final partition ranges with no shift.
- Softmax normalization: reciprocals of the two rowsums -> one bf16 tile
  (rows 0 and 64), two 1-row broadcast matmuls fill a [128,512] psum with
  per-column reciprocals for both heads, then two DVE multiplies.
- Causal masking: diagonal-straddling 128-k-blocks restrict their q range
  to [off:512] (off = 0,128,256,256) and add a bf16 triangle/band mask via
  a small matmul; fully-masked blocks are never computed.
"""
import numpy as np
import ml_dtypes

import concourse.bacc as bacc
import concourse.bass as bass
import concourse.tile as tile
import concourse.mybir as mybir
from concourse.bass_utils import run_bass_kernel_spmd

B, S, D, = 4, 2048, 1024
DC = 512           # per-core d slice (8 heads x 64)
H = 8              # heads per core
DH = 64
N_CORES = 8
F32 = mybir.dt.float32
BF16 = mybir.dt.bfloat16
AF = mybir.ActivationFunctionType
NEG = -1.0e9
SCALE = 1.0 / 32.0  # 1/sqrt(D_MODEL)

# q-range starts for the 4 diagonal-straddling k-blocks of each 512-q chunk
# (last is 256, not 384, to keep matmul moving size >= 256)
OFFS = (0, 128, 256, 256)

_CACHE = {}


def _build():
    nc = bacc.Bacc("TRN2", target_bir_lowering=False, debug=False,
                   num_devices=N_CORES)
    xt = nc.dram_tensor("xt", [D, S], BF16, kind="ExternalInput")
    wq = nc.dram_tensor("wq", [D, DC], BF16, kind="ExternalInput")
    wk = nc.dram_tensor("wk", [D, DC], BF16, kind="ExternalInput")
    wv = nc.dram_tensor("wv", [D, DC], BF16, kind="ExternalInput")
    qb = nc.dram_tensor("qb", [DC], F32, kind="ExternalInput")
    kb = nc.dram_tensor("kb", [DC], F32, kind="ExternalInput")
    wd = nc.dram_tensor("wd", [DC, D], BF16, kind="ExternalInput")
    band = nc.dram_tensor("band", [128, 256], BF16, kind="ExternalInput")
    idm = nc.dram_tensor("idm", [128, 128], BF16, kind="ExternalInput")
    onb = nc.dram_tensor("onb", [128, 128], BF16, kind="ExternalInput")
    out = nc.dram_tensor("out", [S, D], F32, kind="ExternalOutput")

    with tile.TileContext(nc) as tc:
      with nc.allow_low_precision(reason="bf16 storage; all matmul accumulation in fp32 psum"):
        with (
            tc.tile_pool(name="consts", bufs=1) as consts,
            tc.tile_pool(name="ktp", bufs=1) as ktp,
            tc.tile_pool(name="vap", bufs=1) as vap,
            tc.tile_pool(name="otp", bufs=1) as otp,
            tc.tile_pool(name="qtp", bufs=1) as qtp,
            tc.tile_pool(name="xts", bufs=4) as xtsp,
            tc.tile_pool(name="ptp", bufs=2) as ptp,
            tc.tile_pool(name="nrm", bufs=2) as nrm,
            tc.tile_pool(name="psb", bufs=2, space="PSUM") as psb,
            tc.tile_pool(name="psv", bufs=1, space="PSUM") as psv,
            tc.tile_pool(name="psm", bufs=1, space="PSUM") as psm,
        ):
            band_sb = consts.tile([128, 256], BF16)
            nc.gpsimd.dma_start(out=band_sb, in_=band[:, :])
            id_sb = consts.tile([128, 128], BF16)
            nc.gpsimd.dma_start(out=id_sb, in_=idm[:, :])
            onb_sb = consts.tile([128, 128], BF16)
            nc.gpsimd.dma_start(out=onb_sb, in_=onb[:, :])
            qb_sb = consts.tile([128, 4], F32)
            nc.gpsimd.dma_start(out=qb_sb, in_=qb.ap().rearrange("(c p) -> p c", p=128))
            kb_sb = consts.tile([128, 4], F32)
            nc.gpsimd.dma_start(out=kb_sb, in_=kb.ap().rearrange("(c p) -> p c", p=128))

            kt = ktp.tile([128, 4, S], BF16)       # K^T, pair p rows = d 128p..
            qt = qtp.tile([128, 4, S], BF16)       # Q^T, full sequence
            # V per s-block & head pair: [V_A(64) | onesA | onesB | 0*63 | V_B(64)]
            va = vap.tile([128, 16, 4, 193], BF16)
            ot = otp.tile([128, 4, S], BF16)       # O^T accumulated
            nc.vector.memset(va[:, :, :, 64:66], 1.0)
            nc.vector.memset(va[:, :, :, 66:129], 0.0)

            # ---- Phase 1: Q^T, K^T, V projections (stream x^T by s-chunk) ----
            with tc.tile_pool(name="wkv", bufs=1) as wkvp:
                wk_sb = wkvp.tile([128, 8, DC], BF16)
                wq_sb = wkvp.tile([128, 8, DC], BF16)
                wv_sb = wkvp.tile([128, 8, DC], BF16)
                wkv = wk.ap().rearrange("(c p) d -> p c d", p=128)
                wqv = wq.ap().rearrange("(c p) d -> p c d", p=128)
                wvv = wv.ap().rearrange("(c p) d -> p c d", p=128)
                # sliced weight loads so the first matmuls start early
                for i in range(8):
                    nc.scalar.dma_start(out=wk_sb[:, i:i + 1, :], in_=wkv[:, i:i + 1, :])
                for i in range(8):
                    nc.scalar.dma_start(out=wq_sb[:, i:i + 1, :], in_=wqv[:, i:i + 1, :])
                for i in range(8):
                    nc.scalar.dma_start(out=wv_sb[:, i:i + 1, :], in_=wvv[:, i:i + 1, :])

                xv = xt.ap().rearrange("(i p) s -> p i s", p=128)
                xs = {}
                for sc in range(4):
                    x = xtsp.tile([128, 8, 512], BF16, tag="xts", name=f"xts{sc}")
                    if sc == 0:
                        for i in range(8):
                            nc.sync.dma_start(out=x[:, i:i + 1, :],
                                              in_=xv[:, i:i + 1, 0:512])
                    else:
                        for i in range(4):
                            nc.sync.dma_start(
                                out=x[:, 2 * i:2 * i + 2, :],
                                in_=xv[:, 2 * i:2 * i + 2, 512 * sc:512 * (sc + 1)])
                    xs[sc] = x

                for sc in range(4):
                    xg = xs[sc]
                    for p in range(4):
                        ps = psv.tile([128, 512], F32, tag="pvA", bufs=2, name="kps")
                        for i in range(8):
                            nc.tensor.matmul(ps, wk_sb[:, i, 128 * p:128 * (p + 1)],
                                             xg[:, i, :], start=(i == 0), stop=(i == 7))
                        nc.vector.tensor_scalar_add(
                            out=kt[:, p, 512 * sc:512 * (sc + 1)], in0=ps,
                            scalar1=kb_sb[:, p:p + 1])
                    for p in range(4):
                        ps = psv.tile([128, 512], F32, tag="pvA", bufs=2, name="qps")
                        for i in range(8):
                            nc.tensor.matmul(ps, wq_sb[:, i, 128 * p:128 * (p + 1)],
                                             xg[:, i, :], start=(i == 0), stop=(i == 7))
                        nc.vector.tensor_scalar_add(
                            out=qt[:, p, 512 * sc:512 * (sc + 1)], in0=ps,
                            scalar1=qb_sb[:, p:p + 1])
                    for sb_ in range(4):
                        ps = psv.tile([128, 512], F32, tag="pvA", bufs=2, name="vps")
                        for i in range(8):
                            nc.tensor.matmul(ps, xg[:, i, 128 * sb_:128 * (sb_ + 1)],
                                             wv_sb[:, i, :], start=(i == 0), stop=(i == 7))
                        sblk = 4 * sc + sb_
                        pv2 = ps.rearrange("s (pp two d) -> s pp two d", pp=4, two=2)
                        nc.vector.tensor_copy(out=va[:, sblk, :, 0:64], in_=pv2[:, :, 0, :])
                        nc.vector.tensor_copy(out=va[:, sblk, :, 129:193], in_=pv2[:, :, 1, :])

            # ---- Phase 2: attention + dense, chunk at a time ----
            with (
                tc.tile_pool(name="wdp", bufs=1) as wdp,
                tc.tile_pool(name="outp", bufs=3) as outp,
            ):
                wd_sb = wdp.tile([128, 4, D], BF16)
                wdv = wd.ap().rearrange("(c p) d -> p c d", p=128)
                for i in range(4):
                    nc.scalar.dma_start(out=wd_sb[:, i:i + 1, :], in_=wdv[:, i:i + 1, :])
                for c in range(4):
                    nj = 4 * c + 4
                    # diagonal-straddling blocks first: the jj=0 (full-width)
                    # matmul must open every psum accumulation column group
                    order = list(range(4 * c, 4 * c + 4)) + list(range(4 * c))
                    for p in range(4):
                        pvA = psv.tile([65, 512], F32, tag="pvA", bufs=2, name="pvA")
                        pvB = psv.tile([128, 512], F32, tag="pvB", bufs=1, name="pvB")
                        for idx, j in enumerate(order):
                            jj = j - 4 * c
                            diag = jj >= 0
                            off = OFFS[jj] if diag else 0
                            first, last = idx == 0, idx == nj - 1
                            sc_ps = psb.tile([128, 1024], F32)
                            nc.tensor.matmul(sc_ps[:, off:512],
                                             kt[0:64, p, 128 * j:128 * (j + 1)],
                                             qt[0:64, p, 512 * c + off:512 * (c + 1)],
                                             start=True, stop=not diag)
                            nc.tensor.matmul(sc_ps[:, 512 + off:1024],
                                             kt[64:128, p, 128 * j:128 * (j + 1)],
                                             qt[64:128, p, 512 * c + off:512 * (c + 1)],
                                             start=True, stop=not diag)
                            if diag:
                                if jj < 3:
                                    rh, tp0, tpw = band_sb[:, 128:256], 128 * jj, 128
                                else:
                                    rh, tp0, tpw = band_sb[:, 0:256], 256, 256
                                nc.tensor.matmul(sc_ps[:, tp0:tp0 + tpw], id_sb, rh,
                                                 start=False, stop=True)
                                nc.tensor.matmul(sc_ps[:, 512 + tp0:512 + tp0 + tpw],
                                                 id_sb, rh, start=False, stop=True)
                            pt = ptp.tile([128, 1024], BF16)
                            if off:
                                sc3 = sc_ps.rearrange("p (h q) -> p h q", h=2)[:, :, off:512]
                                pt3 = pt.rearrange("p (h q) -> p h q", h=2)[:, :, off:512]
                                nc.scalar.activation(out=pt3, in_=sc3, func=AF.Exp,
                                                     scale=SCALE)
                            else:
                                nc.scalar.activation(out=pt, in_=sc_ps, func=AF.Exp,
                                                     scale=SCALE)
                            nc.tensor.matmul(pvA[:, off:512], va[:, j, p, 0:65],
                                             pt[:, off:512], start=first, stop=last)
                            nc.tensor.matmul(pvB[:, off:512], va[:, j, p, 65:193],
                                             pt[:, 512 + off:1024], start=first, stop=last)
                        # normalization: rowsum_A at pvA[64], rowsum_B at pvB[0]
                        rr2 = nrm.tile([65, 512], BF16, tag="rr")
                        nc.vector.reciprocal(out=rr2[64:65, :], in_=pvA[64:65, :])
                        nc.vector.reciprocal(out=rr2[0:1, :], in_=pvB[0:1, :])
                        bc = psm.tile([128, 512], F32, tag="mm", name="bc")
                        nc.tensor.matmul(bc[0:64, :], onb_sb[64:65, 0:64], rr2[64:65, :],
                                         start=True, stop=True, tile_position=(64, 0))
                        nc.tensor.matmul(bc[64:128, :], onb_sb[0:1, 64:128], rr2[0:1, :],
                                         start=True, stop=True, tile_position=(0, 64))
                        # DVE cannot read two PSUM operands in one op: stage bc
                        bcs = nrm.tile([128, 512], BF16, tag="bcs")
                        nc.vector.tensor_copy(out=bcs, in_=bc)
                        nc.vector.tensor_mul(out=ot[0:64, p, 512 * c:512 * (c + 1)],
                                             in0=pvA[0:64, :], in1=bcs[0:64, :])
                        nc.vector.tensor_mul(out=ot[64:128, p, 512 * c:512 * (c + 1)],
                                             in0=pvB[64:128, :], in1=bcs[64:128, :])
                    # dense for this chunk's 4 s-blocks
                    for sb_ in range(4 * c, 4 * c + 4):
                        os = outp.tile([128, 1024], F32)
                        for n in range(2):
                            ps = psv.tile([128, 512], F32, tag="pvA", bufs=2, name="dps")
                            for p in range(4):
                                nc.tensor.matmul(ps, ot[:, p, 128 * sb_:128 * (sb_ + 1)],
                                                 wd_sb[:, p, 512 * n:512 * (n + 1)],
                                                 start=(p == 0), stop=(p == 3))
                            nc.vector.tensor_copy(out=os[:, 512 * n:512 * (n + 1)], in_=ps)
                        nc.sync.dma_start(out=out[128 * sb_:128 * (sb_ + 1), :], in_=os)
    nc.compile()
    return nc


def get_nc():
    if "nc" not in _CACHE:
        _CACHE["nc"] = _build()
    return _CACHE["nc"]


def kernel(x, mask, wq_w, wq_b, wk_w, wk_b, wv_w, wv_b, dense_w, dense_b,
           _trace=False):
    bf = ml_dtypes.bfloat16
    x = np.asarray(x, dtype=np.float32)
    wq_w = np.asarray(wq_w, np.float32); wq_b = np.asarray(wq_b, np.float32)
    wk_w = np.asarray(wk_w, np.float32); wk_b = np.asarray(wk_b, np.float32)
    wv_w = np.asarray(wv_w, np.float32); wv_b = np.asarray(wv_b, np.float32)
    dense_w = np.asarray(dense_w, np.float32)
    dense_b = np.asarray(dense_b, np.float32)

    # causal masks, bf16: cols 0:128 = all -1e9; cols 128:256 = triangle
    # T[k, qq] = -1e9 where qq < k
    band = np.zeros((128, 256), np.float32)
    band[:, 0:128] = NEG
    k_idx = np.arange(128)[:, None]
    q_idx = np.arange(128)[None, :]
    band[:, 128:256] = np.where(q_idx < k_idx, NEG, 0.0)
    ident = np.eye(128, dtype=np.float32)
    onb = np.zeros((128, 128), np.float32)
    onb[64, 0:64] = 1.0   # broadcasts rowsum_A recip (at partition 64) to rows 0:64
    onb[0, 64:128] = 1.0  # broadcasts rowsum_B recip (at partition 0) to rows 64:128

    in_maps = []
    for core in range(N_CORES):
        b, hh = divmod(core, 2)
        sl = slice(DC * hh, DC * (hh + 1))
        in_maps.append({
            "xt": np.ascontiguousarray(x[b].T).astype(bf),
            "wq": np.ascontiguousarray(wq_w[:, sl]).astype(bf),
            "wk": np.ascontiguousarray(wk_w[:, sl]).astype(bf),
            "wv": np.ascontiguousarray(wv_w[:, sl]).astype(bf),
            "qb": np.ascontiguousarray(wq_b[sl]),
            "kb": np.ascontiguousarray(wk_b[sl]),
            "wd": np.ascontiguousarray(dense_w[sl, :]).astype(bf),
            "band": band.astype(bf), "idm": ident.astype(bf),
            "onb": onb.astype(bf),
        })
    nc = get_nc()
    res = run_bass_kernel_spmd(nc, in_maps, core_ids=list(range(N_CORES)),
                               trace=_trace)
    const = dense_b + wv_b @ dense_w  # bias terms deferred to host
    outs = np.empty((B, S, D), np.float32)
    for b in range(B):
        outs[b] = res.results[2 * b]["out"] + res.results[2 * b + 1]["out"] + const
    if _trace:
        kernel.last_result = res
    return outs


# revision 4
# speedup vs baseline: 1.2378x; 1.0611x over previous
"""Causal MHA forward on 8 NeuronCores (Trainium2, Bass/Tile).

Sharding: batch (4) x head-half (2) -> 8 cores. Each core computes, for its
batch b and its 8 heads: QKV column-sliced projections (bf16), causal
attention in transposed-score layout (S^T[k, q]), and a partial dense
projection against the matching 512-row slice of dense_w. The host sums the
two partial dense outputs per batch and adds dense_b + wv_b @ dense_w
(valid because softmax rows sum to 1).

Key layout tricks:
- Scores/PV computed per head-pair p (heads 2p, 2p+1). Head 2p's PV lhsT is
  [V_A | ones] -> psum rows 0:64 = O_A, row 64 = rowsum_A. Head 2p+1's lhsT
  is [ones | zeros*63 | V_B] -> psum row 0 = rowsum_B, rows 64:128 = O_B, so
  both heads' outputs land at their final partition ranges with no shift.
- Softmax normalization: reciprocals of the two rowsums -> one bf16 tile
  (rows 0 and 64), two 1-row broadcast matmuls fill a [128,512] psum with
  per-column reciprocals for both heads, then two DVE multiplies.
- Causal masking: diagonal-straddling 128-k-blocks restrict their q range
  to [off:512] (off = 0,128,256,256) and add a bf16 triangle/band mask via
  a small matmul; fully-masked blocks are never computed.
"""
import numpy as np
import ml_dtypes

import concourse.bacc as bacc
import concourse.bass as bass
import concourse.tile as tile
import concourse.mybir as mybir
from concourse.bass_utils import run_bass_kernel_spmd

B, S, D, = 4, 2048, 1024
DC = 512           # per-core d slice (8 heads x 64)
H = 8              # heads per core
DH = 64
N_CORES = 8
F32 = mybir.dt.float32
BF16 = mybir.dt.bfloat16
AF = mybir.ActivationFunctionType
NEG = -1.0e9
SCALE = 1.0 / 32.0  # 1/sqrt(D_MODEL)

# q-range starts for the 4 diagonal-straddling k-blocks of each 512-q chunk
# (last is 256, not 384, to keep matmul moving size >= 256)
OFFS = (0, 128, 256, 256)

_CACHE = {}


def _build():
    nc = bacc.Bacc("TRN2", target_bir_lowering=False, debug=False,
                   num_devices=N_CORES)
    xt = nc.dram_tensor("xt", [D, S], BF16, kind="ExternalInput")
    wq = nc.dram_tensor("wq", [D, DC], BF16, kind="ExternalInput")
    wk = nc.dram_tensor("wk", [D, DC], BF16, kind="ExternalInput")
    wv = nc.dram_tensor("wv", [D, DC], BF16, kind="ExternalInput")
    qb = nc.dram_tensor("qb", [DC], F32, kind="ExternalInput")
    kb = nc.dram_tensor("kb", [DC], F32, kind="ExternalInput")
    wd = nc.dram_tensor("wd", [DC, D], BF16, kind="ExternalInput")
    band = nc.dram_tensor("band", [128, 256], BF16, kind="ExternalInput")
    idm = nc.dram_tensor("idm", [128, 128], BF16, kind="ExternalInput")
    onb = nc.dram_tensor("onb", [128, 128], BF16, kind="ExternalInput")
    out = nc.dram_tensor("out", [S, D], F32, kind="ExternalOutput")

    with tile.TileContext(nc) as tc:
      with nc.allow_low_precision(reason="bf16 storage; all matmul accumulation in fp32 psum"):
        with (
            tc.tile_pool(name="consts", bufs=1) as consts,
            tc.tile_pool(name="ktp", bufs=1) as ktp,
            tc.tile_pool(name="vap", bufs=1) as vap,
            tc.tile_pool(name="otp", bufs=1) as otp,
            tc.tile_pool(name="qtp", bufs=1) as qtp,
            tc.tile_pool(name="xts", bufs=4) as xtsp,
            tc.tile_pool(name="ptp", bufs=2) as ptp,
            tc.tile_pool(name="nrm", bufs=2) as nrm,
            tc.tile_pool(name="psb", bufs=2, space="PSUM") as psb,
            tc.tile_pool(name="psv", bufs=1, space="PSUM") as psv,
            tc.tile_pool(name="psm", bufs=1, space="PSUM") as psm,
        ):
            band_sb = consts.tile([128, 256], BF16)
            nc.gpsimd.dma_start(out=band_sb, in_=band[:, :])
            id_sb = consts.tile([128, 128], BF16)
            nc.gpsimd.dma_start(out=id_sb, in_=idm[:, :])
            onb_sb = consts.tile([128, 128], BF16)
            nc.gpsimd.dma_start(out=onb_sb, in_=onb[:, :])
            qb_sb = consts.tile([128, 4], F32)
            nc.gpsimd.dma_start(out=qb_sb, in_=qb.ap().rearrange("(c p) -> p c", p=128))
            kb_sb = consts.tile([128, 4], F32)
            nc.gpsimd.dma_start(out=kb_sb, in_=kb.ap().rearrange("(c p) -> p c", p=128))

            kt = ktp.tile([128, 4, S], BF16)       # K^T, pair p rows = d 128p..
            qt = qtp.tile([128, 4, S], BF16)       # Q^T, full sequence
            # V per s-block & head pair: [V_A(64) | onesA | onesB | 0*63 | V_B(64)]
            va = vap.tile([128, 16, 4, 193], BF16)
            ot = otp.tile([128, 4, S], BF16)       # O^T accumulated
            nc.vector.memset(va[:, :, :, 64:66], 1.0)
            nc.vector.memset(va[:, :, :, 66:129], 0.0)

            # ---- Phase 1: Q^T, K^T, V projections (stream x^T by s-chunk) ----
            with tc.tile_pool(name="wkv", bufs=1) as wkvp:
                wk_sb = wkvp.tile([128, 8, DC], BF16)
                wq_sb = wkvp.tile([128, 8, DC], BF16)
                wv_sb = wkvp.tile([128, 8, DC], BF16)
                wkv = wk.ap().rearrange("(c p) d -> p c d", p=128)
                wqv = wq.ap().rearrange("(c p) d -> p c d", p=128)
                wvv = wv.ap().rearrange("(c p) d -> p c d", p=128)
                # sliced weight loads so the first matmuls start early
                for i in range(8):
                    nc.scalar.dma_start(out=wk_sb[:, i:i + 1, :], in_=wkv[:, i:i + 1, :])
                for i in range(8):
                    nc.scalar.dma_start(out=wq_sb[:, i:i + 1, :], in_=wqv[:, i:i + 1, :])
                for i in range(8):
                    nc.scalar.dma_start(out=wv_sb[:, i:i + 1, :], in_=wvv[:, i:i + 1, :])

                xv = xt.ap().rearrange("(i p) s -> p i s", p=128)
                xs = {}
                for sc in range(4):
                    x = xtsp.tile([128, 8, 512], BF16, tag="xts", name=f"xts{sc}")
                    if sc == 0:
                        for i in range(8):
                            nc.sync.dma_start(out=x[:, i:i + 1, :],
                                              in_=xv[:, i:i + 1, 0:512])
                    else:
                        for i in range(4):
                            nc.sync.dma_start(
                                out=x[:, 2 * i:2 * i + 2, :],
                                in_=xv[:, 2 * i:2 * i + 2, 512 * sc:512 * (sc + 1)])
                    xs[sc] = x

                for sc in range(4):
                    xg = xs[sc]
                    for p in range(4):
                        ps = psv.tile([128, 512], F32, tag="pvA", bufs=2, name="kps")
                        for i in range(8):
                            nc.tensor.matmul(ps, wk_sb[:, i, 128 * p:128 * (p + 1)],
                                             xg[:, i, :], start=(i == 0), stop=(i == 7))
                        nc.vector.tensor_scalar_add(
                            out=kt[:, p, 512 * sc:512 * (sc + 1)], in0=ps,
                            scalar1=kb_sb[:, p:p + 1])
                    for p in range(4):
                        ps = psv.tile([128, 512], F32, tag="pvA", bufs=2, name="qps")
                        for i in range(8):
                            nc.tensor.matmul(ps, wq_sb[:, i, 128 * p:128 * (p + 1)],
                                             xg[:, i, :], start=(i == 0), stop=(i == 7))
                        nc.vector.tensor_scalar_add(
                            out=qt[:, p, 512 * sc:512 * (sc + 1)], in0=ps,
                            scalar1=qb_sb[:, p:p + 1])
                    for sb_ in range(4):
                        ps = psv.tile([128, 512], F32, tag="pvA", bufs=2, name="vps")
                        for i in range(8):
                            nc.tensor.matmul(ps, xg[:, i, 128 * sb_:128 * (sb_ + 1)],
                                             wv_sb[:, i, :], start=(i == 0), stop=(i == 7))
                        sblk = 4 * sc + sb_
                        pv2 = ps.rearrange("s (pp two d) -> s pp two d", pp=4, two=2)
                        nc.vector.tensor_copy(out=va[:, sblk, :, 0:64], in_=pv2[:, :, 0, :])
                        nc.vector.tensor_copy(out=va[:, sblk, :, 129:193], in_=pv2[:, :, 1, :])

            # ---- Phase 2: attention + dense, chunk at a time ----
            with (
                tc.tile_pool(name="wdp", bufs=1) as wdp,
                tc.tile_pool(name="outp", bufs=3) as outp,
            ):
                wd_sb = wdp.tile([128, 4, D], BF16)
                wdv = wd.ap().rearrange("(c p) d -> p c d", p=128)
                for i in range(4):
                    nc.scalar.dma_start(out=wd_sb[:, i:i + 1, :], in_=wdv[:, i:i + 1, :])
                for c in range(4):
                    nj = 4 * c + 4
                    # diagonal-straddling blocks first: the jj=0 (full-width)
                    # matmul must open every psum accumulation column group
                    order = list(range(4 * c, 4 * c + 4)) + list(range(4 * c))
                    for p in range(4):
                        pvA = psv.tile([65, 512], F32, tag="pvA", bufs=2, name="pvA")
                        pvB = psv.tile([128, 512], F32, tag="pvB", bufs=1, name="pvB")
                        for idx, j in enumerate(order):
                            jj = j - 4 * c
                            diag = jj >= 0
                            off = OFFS[jj] if diag else 0
                            first, last = idx == 0, idx == nj - 1
                            sc_ps = psb.tile([128, 1024], F32)
                            nc.tensor.matmul(sc_ps[:, off:512],
                                             kt[0:64, p, 128 * j:128 * (j + 1)],
                                             qt[0:64, p, 512 * c + off:512 * (c + 1)],
                                             start=True, stop=not diag)
                            nc.tensor.matmul(sc_ps[:, 512 + off:1024],
                                             kt[64:128, p, 128 * j:128 * (j + 1)],
                                             qt[64:128, p, 512 * c + off:512 * (c + 1)],
                                             start=True, stop=not diag)
                            if diag:
                                if jj < 3:
                                    rh, tp0, tpw = band_sb[:, 128:256], 128 * jj, 128
                                else:
                                    rh, tp0, tpw = band_sb[:, 0:256], 256, 256
                                nc.tensor.matmul(sc_ps[:, tp0:tp0 + tpw], id_sb, rh,
                                                 start=False, stop=True)
                                nc.tensor.matmul(sc_ps[:, 512 + tp0:512 + tp0 + tpw],
                                                 id_sb, rh, start=False, stop=True)
                            pt = ptp.tile([128, 1024], BF16)
                            if off:
                                sc3 = sc_ps.rearrange("p (h q) -> p h q", h=2)[:, :, off:512]
                                pt3 = pt.rearrange("p (h q) -> p h q", h=2)[:, :, off:512]
                                nc.scalar.activation(out=pt3, in_=sc3, func=AF.Exp,
                                                     scale=SCALE)
                            else:
                                nc.scalar.activation(out=pt, in_=sc_ps, func=AF.Exp,
                                                     scale=SCALE)
                            nc.tensor.matmul(pvA[:, off:512], va[:, j, p, 0:65],
                                             pt[:, off:512], start=first, stop=last)
                            nc.tensor.matmul(pvB[:, off:512], va[:, j, p, 65:193],
                                             pt[:, 512 + off:1024], start=first, stop=last)
                        # normalization: rowsum_A at pvA[64], rowsum_B at pvB[0].
                        # Copy both psums to SBUF first so the banks free fast
                        # (next head-pair's PV matmuls reuse them), then work
                        # off the copies.
                        pvAc = nrm.tile([65, 512], BF16, tag="pvAc")
                        pvBc = nrm.tile([128, 512], BF16, tag="pvBc")
                        nc.vector.tensor_copy(out=pvAc, in_=pvA)
                        nc.vector.tensor_copy(out=pvBc, in_=pvB)
                        rr2 = nrm.tile([65, 512], BF16, tag="rr")
                        nc.vector.reciprocal(out=rr2[64:65, :], in_=pvAc[64:65, :])
                        nc.vector.reciprocal(out=rr2[0:1, :], in_=pvBc[0:1, :])
                        bc = psm.tile([128, 512], F32, tag="mm", name="bc")
                        nc.tensor.matmul(bc[0:64, :], onb_sb[64:65, 0:64], rr2[64:65, :],
                                         start=True, stop=True, tile_position=(64, 0))
                        nc.tensor.matmul(bc[64:128, :], onb_sb[0:1, 64:128], rr2[0:1, :],
                                         start=True, stop=True, tile_position=(0, 64))
                        nc.vector.tensor_mul(out=ot[0:64, p, 512 * c:512 * (c + 1)],
                                             in0=pvAc[0:64, :], in1=bc[0:64, :])
                        nc.vector.tensor_mul(out=ot[64:128, p, 512 * c:512 * (c + 1)],
                                             in0=pvBc[64:128, :], in1=bc[64:128, :])
                    # dense for this chunk's 4 s-blocks
                    for sb_ in range(4 * c, 4 * c + 4):
                        os = outp.tile([128, 1024], F32)
                        for n in range(2):
                            ps = psv.tile([128, 512], F32, tag="pvA", bufs=2, name="dps")
                            for p in range(4):
                                nc.tensor.matmul(ps, ot[:, p, 128 * sb_:128 * (sb_ + 1)],
                                                 wd_sb[:, p, 512 * n:512 * (n + 1)],
                                                 start=(p == 0), stop=(p == 3))
                            nc.vector.tensor_copy(out=os[:, 512 * n:512 * (n + 1)], in_=ps)
                        nc.sync.dma_start(out=out[128 * sb_:128 * (sb_ + 1), :], in_=os)
    nc.compile()
    return nc


def get_nc():
    if "nc" not in _CACHE:
        _CACHE["nc"] = _build()
    return _CACHE["nc"]


def kernel(x, mask, wq_w, wq_b, wk_w, wk_b, wv_w, wv_b, dense_w, dense_b,
           _trace=False):
    bf = ml_dtypes.bfloat16
    x = np.asarray(x, dtype=np.float32)
    wq_w = np.asarray(wq_w, np.float32); wq_b = np.asarray(wq_b, np.float32)
    wk_w = np.asarray(wk_w, np.float32); wk_b = np.asarray(wk_b, np.float32)
    wv_w = np.asarray(wv_w, np.float32); wv_b = np.asarray(wv_b, np.float32)
    dense_w = np.asarray(dense_w, np.float32)
    dense_b = np.asarray(dense_b, np.float32)

    # causal masks, bf16: cols 0:128 = all -1e9; cols 128:256 = triangle
    # T[k, qq] = -1e9 where qq < k
    band = np.zeros((128, 256), np.float32)
    band[:, 0:128] = NEG
    k_idx = np.arange(128)[:, None]
    q_idx = np.arange(128)[None, :]
    band[:, 128:256] = np.where(q_idx < k_idx, NEG, 0.0)
    ident = np.eye(128, dtype=np.float32)
    onb = np.zeros((128, 128), np.float32)
    onb[64, 0:64] = 1.0   # broadcasts rowsum_A recip (at partition 64) to rows 0:64
    onb[0, 64:128] = 1.0  # broadcasts rowsum_B recip (at partition 0) to rows 64:128

    in_maps = []
    for core in range(N_CORES):
        b, hh = divmod(core, 2)
        sl = slice(DC * hh, DC * (hh + 1))
        in_maps.append({
            "xt": np.ascontiguousarray(x[b].T).astype(bf),
            "wq": np.ascontiguousarray(wq_w[:, sl]).astype(bf),
            "wk": np.ascontiguousarray(wk_w[:, sl]).astype(bf),
            "wv": np.ascontiguousarray(wv_w[:, sl]).astype(bf),
            "qb": np.ascontiguousarray(wq_b[sl]),
            "kb": np.ascontiguousarray(wk_b[sl]),
            "wd": np.ascontiguousarray(dense_w[sl, :]).astype(bf),
            "band": band.astype(bf), "idm": ident.astype(bf),
            "onb": onb.astype(bf),
        })
    nc = get_nc()
    res = run_bass_kernel_spmd(nc, in_maps, core_ids=list(range(N_CORES)),
                               trace=_trace)
    const = dense_b + wv_b @ dense_w  # bias terms deferred to host
    outs = np.empty((B, S, D), np.float32)
    for b in range(B):
        outs[b] = res.results[2 * b]["out"] + res.results[2 * b + 1]["out"] + const
    if _trace:
        kernel.last_result = res
    return outs


# revision 10
# speedup vs baseline: 1.2976x; 1.0483x over previous
"""Causal MHA forward on 8 NeuronCores (Trainium2, Bass/Tile).

Sharding: batch (4) x head-half (2) -> 8 cores. Each core computes, for its
batch b and its 8 heads: QKV column-sliced projections (bf16), causal
attention in transposed-score layout (S^T[k, q]), and a partial dense
projection against the matching 512-row slice of dense_w. The host sums the
two partial dense outputs per batch and adds dense_b + wv_b @ dense_w
(valid because softmax rows sum to 1).

Key layout tricks:
- Scores/PV computed per head-pair p (heads 2p, 2p+1). Head 2p's PV lhsT is
  [V_A | ones] -> psum rows 0:64 = O_A, row 64 = rowsum_A. Head 2p+1's lhsT
  is [ones | zeros*63 | V_B] -> psum row 0 = rowsum_B, rows 64:128 = O_B, so
  both heads' outputs land at their final partition ranges with no shift.
- Softmax normalization: reciprocals of the two rowsums -> one bf16 tile
  (rows 0 and 64), two 1-row broadcast matmuls fill a [128,512] psum with
  per-column reciprocals for both heads, then two DVE multiplies.
- Causal masking: diagonal-straddling 128-k-blocks restrict their q range
  to [off:512] (off = 0,128,256,256) and add a bf16 triangle/band mask via
  a small matmul; fully-masked blocks are never computed.
"""
import numpy as np
import ml_dtypes

import concourse.bacc as bacc
import concourse.bass as bass
import concourse.tile as tile
import concourse.mybir as mybir
from concourse.bass_utils import run_bass_kernel_spmd

B, S, D, = 4, 2048, 1024
DC = 512           # per-core d slice (8 heads x 64)
H = 8              # heads per core
DH = 64
N_CORES = 8
F32 = mybir.dt.float32
BF16 = mybir.dt.bfloat16
AF = mybir.ActivationFunctionType
NEG = -1.0e9
SCALE = 1.0 / 32.0  # 1/sqrt(D_MODEL)

# q-range starts for the 4 diagonal-straddling k-blocks of each 512-q chunk
# (last is 256, not 384, to keep matmul moving size >= 256)
OFFS = (0, 128, 256, 256)

_CACHE = {}


def _build():
    nc = bacc.Bacc("TRN2", target_bir_lowering=False, debug=False,
                   num_devices=N_CORES)
    xt = nc.dram_tensor("xt", [D, S], BF16, kind="ExternalInput")
    wq = nc.dram_tensor("wq", [D, DC], BF16, kind="ExternalInput")
    wk = nc.dram_tensor("wk", [D, DC], BF16, kind="ExternalInput")
    wv = nc.dram_tensor("wv", [D, DC], BF16, kind="ExternalInput")
    qb = nc.dram_tensor("qb", [DC], F32, kind="ExternalInput")
    kb = nc.dram_tensor("kb", [DC], F32, kind="ExternalInput")
    wd = nc.dram_tensor("wd", [DC, D], BF16, kind="ExternalInput")
    band = nc.dram_tensor("band", [128, 256], BF16, kind="ExternalInput")
    idm = nc.dram_tensor("idm", [128, 128], BF16, kind="ExternalInput")
    onb = nc.dram_tensor("onb", [128, 128], BF16, kind="ExternalInput")
    out = nc.dram_tensor("out", [S, D], F32, kind="ExternalOutput")

    with tile.TileContext(nc) as tc:
      with nc.allow_low_precision(reason="bf16 storage; all matmul accumulation in fp32 psum"):
        with (
            tc.tile_pool(name="consts", bufs=1) as consts,
            tc.tile_pool(name="ktp", bufs=1) as ktp,
            tc.tile_pool(name="vap", bufs=1) as vap,
            tc.tile_pool(name="otp", bufs=1) as otp,
            tc.tile_pool(name="qtp", bufs=1) as qtp,
            tc.tile_pool(name="xts", bufs=4) as xtsp,
            tc.tile_pool(name="ptp", bufs=2) as ptp,
            tc.tile_pool(name="nrm", bufs=2) as nrm,
            tc.tile_pool(name="psb", bufs=2, space="PSUM") as psb,
            tc.tile_pool(name="psv", bufs=1, space="PSUM") as psv,
            tc.tile_pool(name="psm", bufs=1, space="PSUM") as psm,
            tc.tile_pool(name="wts", bufs=1) as wkvp,
            tc.tile_pool(name="outp", bufs=3) as outp,
        ):
            band_sb = consts.tile([128, 256], BF16)
            nc.gpsimd.dma_start(out=band_sb, in_=band[:, :])
            id_sb = consts.tile([128, 128], BF16)
            nc.gpsimd.dma_start(out=id_sb, in_=idm[:, :])
            onb_sb = consts.tile([128, 128], BF16)
            nc.gpsimd.dma_start(out=onb_sb, in_=onb[:, :])
            qb_sb = consts.tile([128, 4], F32)
            nc.gpsimd.dma_start(out=qb_sb, in_=qb.ap().rearrange("(c p) -> p c", p=128))
            kb_sb = consts.tile([128, 4], F32)
            nc.gpsimd.dma_start(out=kb_sb, in_=kb.ap().rearrange("(c p) -> p c", p=128))

            kt = ktp.tile([128, 4, S], BF16)       # K^T, pair p rows = d 128p..
            qt = qtp.tile([128, 4, S], BF16)       # Q^T, full sequence
            # V per s-block & head pair: [V_A(64) | onesA | onesB | 0*63 | V_B(64)]
            va = vap.tile([128, 16, 4, 193], BF16)
            ot = otp.tile([128, 4, S], BF16)       # O^T accumulated
            nc.vector.memset(va[:, :, :, 64:66], 1.0)
            nc.vector.memset(va[:, :, :, 66:129], 0.0)

            # persistent normalization rhs: rows 0 (1/rowsum_B) and 64
            # (1/rowsum_A) are rewritten per head-pair; rows 1:64 stay zero so
            # a single K=65 broadcast matmul can read the whole tile
            rr2 = nrm.tile([65, 512], BF16, tag="rrP")
            nc.vector.memset(rr2, 0.0)

            wk_sb = wkvp.tile([128, 8, DC], BF16)
            wq_sb = wkvp.tile([128, 8, DC], BF16)
            wv_sb = wkvp.tile([128, 8, DC], BF16)
            wd_sb = wkvp.tile([128, 4, D], BF16)
            wkv = wk.ap().rearrange("(c p) d -> p c d", p=128)
            wqv = wq.ap().rearrange("(c p) d -> p c d", p=128)
            wvv = wv.ap().rearrange("(c p) d -> p c d", p=128)
            wdv = wd.ap().rearrange("(c p) d -> p c d", p=128)
            # sliced weight loads so the first matmuls start early; order
            # matches phase-1 compute order K, V, Q
            for i in range(8):
                nc.scalar.dma_start(out=wk_sb[:, i:i + 1, :], in_=wkv[:, i:i + 1, :])
            for i in range(8):
                nc.scalar.dma_start(out=wv_sb[:, i:i + 1, :], in_=wvv[:, i:i + 1, :])
            for i in range(8):
                nc.scalar.dma_start(out=wq_sb[:, i:i + 1, :], in_=wqv[:, i:i + 1, :])
            for i in range(4):
                nc.scalar.dma_start(out=wd_sb[:, i:i + 1, :], in_=wdv[:, i:i + 1, :])

            # ---- Phase 1: K^T, V, Q^T projections (stream x^T by s-chunk) ----
            xv = xt.ap().rearrange("(i p) s -> p i s", p=128)
            xs = {}
            for sc in range(4):
                x = xtsp.tile([128, 8, 512], BF16, tag="xts", name=f"xts{sc}")
                if sc == 0:
                    for i in range(8):
                        nc.sync.dma_start(out=x[:, i:i + 1, :],
                                          in_=xv[:, i:i + 1, 0:512])
                else:
                    for i in range(4):
                        nc.sync.dma_start(
                            out=x[:, 2 * i:2 * i + 2, :],
                            in_=xv[:, 2 * i:2 * i + 2, 512 * sc:512 * (sc + 1)])
                xs[sc] = x

            for sc in range(4):
                xg = xs[sc]
                for p in range(4):
                    ps = psv.tile([128, 512], F32, tag="pvA", bufs=2, name="kps")
                    for i in range(8):
                        nc.tensor.matmul(ps, wk_sb[:, i, 128 * p:128 * (p + 1)],
                                         xg[:, i, :], start=(i == 0), stop=(i == 7))
                    nc.vector.tensor_scalar_add(
                        out=kt[:, p, 512 * sc:512 * (sc + 1)], in0=ps,
                        scalar1=kb_sb[:, p:p + 1])
                for sb_ in range(4):
                    ps = psv.tile([128, 512], F32, tag="pvA", bufs=2, name="vps")
                    for i in range(8):
                        nc.tensor.matmul(ps, xg[:, i, 128 * sb_:128 * (sb_ + 1)],
                                         wv_sb[:, i, :], start=(i == 0), stop=(i == 7))
                    sblk = 4 * sc + sb_
                    pv2 = ps.rearrange("s (pp two d) -> s pp two d", pp=4, two=2)
                    nc.vector.tensor_copy(out=va[:, sblk, :, 0:64], in_=pv2[:, :, 0, :])
                    nc.vector.tensor_copy(out=va[:, sblk, :, 129:193], in_=pv2[:, :, 1, :])
                for p in range(4):
                    ps = psv.tile([128, 512], F32, tag="pvA", bufs=2, name="qps")
                    for i in range(8):
                        nc.tensor.matmul(ps, wq_sb[:, i, 128 * p:128 * (p + 1)],
                                         xg[:, i, :], start=(i == 0), stop=(i == 7))
                    nc.vector.tensor_scalar_add(
                        out=qt[:, p, 512 * sc:512 * (sc + 1)], in0=ps,
                        scalar1=qb_sb[:, p:p + 1])

            # ---- Phase 2: attention + dense; chunk c-1's dense blocks are
            # interleaved into chunk c's p-loop to fill PE stalls ----
            def dense_block(sb_):
                os = outp.tile([128, 1024], F32)
                for n in range(2):
                    ps = psv.tile([128, 512], F32, tag="pvA", bufs=2, name="dps")
                    for p in range(4):
                        nc.tensor.matmul(ps, ot[:, p, 128 * sb_:128 * (sb_ + 1)],
                                         wd_sb[:, p, 512 * n:512 * (n + 1)],
                                         start=(p == 0), stop=(p == 3))
                    nc.vector.tensor_copy(out=os[:, 512 * n:512 * (n + 1)], in_=ps)
                nc.sync.dma_start(out=out[128 * sb_:128 * (sb_ + 1), :], in_=os)

            if True:
                for c in range(4):
                    nj = 4 * c + 4
                    # diagonal-straddling blocks first: the jj=0 (full-width)
                    # matmul must open every psum accumulation column group
                    order = list(range(4 * c, 4 * c + 4)) + list(range(4 * c))
                    for p in range(4):
                        if c > 0:
                            dense_block(4 * (c - 1) + p)
                        pvA = psv.tile([65, 512], F32, tag="pvA", bufs=2, name="pvA")
                        pvB = psv.tile([128, 512], F32, tag="pvB", bufs=1, name="pvB")
                        for idx, j in enumerate(order):
                            jj = j - 4 * c
                            diag = jj >= 0
                            off = OFFS[jj] if diag else 0
                            first, last = idx == 0, idx == nj - 1
                            sc_ps = psb.tile([128, 1024], F32)
                            nc.tensor.matmul(sc_ps[:, off:512],
                                             kt[0:64, p, 128 * j:128 * (j + 1)],
                                             qt[0:64, p, 512 * c + off:512 * (c + 1)],
                                             start=True, stop=not diag)
                            nc.tensor.matmul(sc_ps[:, 512 + off:1024],
                                             kt[64:128, p, 128 * j:128 * (j + 1)],
                                             qt[64:128, p, 512 * c + off:512 * (c + 1)],
                                             start=True, stop=not diag)
                            if diag:
                                if jj < 3:
                                    rh, tp0, tpw = band_sb[:, 128:256], 128 * jj, 128
                                else:
                                    rh, tp0, tpw = band_sb[:, 0:256], 256, 256
                                nc.tensor.matmul(sc_ps[:, tp0:tp0 + tpw], id_sb, rh,
                                                 start=False, stop=True)
                                nc.tensor.matmul(sc_ps[:, 512 + tp0:512 + tp0 + tpw],
                                                 id_sb, rh, start=False, stop=True)
                            pt = ptp.tile([128, 1024], BF16)
                            if off:
                                sc3 = sc_ps.rearrange("p (h q) -> p h q", h=2)[:, :, off:512]
                                pt3 = pt.rearrange("p (h q) -> p h q", h=2)[:, :, off:512]
                                nc.scalar.activation(out=pt3, in_=sc3, func=AF.Exp,
                                                     scale=SCALE)
                            else:
                                nc.scalar.activation(out=pt, in_=sc_ps, func=AF.Exp,
                                                     scale=SCALE)
                            nc.tensor.matmul(pvA[:, off:512], va[:, j, p, 0:65],
                                             pt[:, off:512], start=first, stop=last)
                            nc.tensor.matmul(pvB[:, off:512], va[:, j, p, 65:193],
                                             pt[:, 512 + off:1024], start=first, stop=last)
                        # normalization: rowsum_A at pvA[64], rowsum_B at pvB[0].
                        # Copy both psums to SBUF first so the banks free fast
                        # (next head-pair's PV matmuls reuse them), then work
                        # off the copies.
                        pvAc = nrm.tile([65, 512], BF16, tag="pvAc")
                        pvBc = nrm.tile([128, 512], BF16, tag="pvBc")
                        nc.vector.tensor_copy(out=pvAc, in_=pvA)
                        nc.vector.tensor_copy(out=pvBc, in_=pvB)
                        nc.vector.reciprocal(out=rr2[64:65, :], in_=pvAc[64:65, :])
                        nc.vector.reciprocal(out=rr2[0:1, :], in_=pvBc[0:1, :])
                        bc = psm.tile([128, 512], F32, tag="mm", name="bc")
                        nc.tensor.matmul(bc, onb_sb[0:65, :], rr2[0:65, :],
                                         start=True, stop=True)
                        nc.vector.tensor_mul(out=ot[0:64, p, 512 * c:512 * (c + 1)],
                                             in0=pvAc[0:64, :], in1=bc[0:64, :])
                        nc.vector.tensor_mul(out=ot[64:128, p, 512 * c:512 * (c + 1)],
                                             in0=pvBc[64:128, :], in1=bc[64:128, :])
                # dense for the last chunk's 4 s-blocks
                for sb_ in range(12, 16):
                    dense_block(sb_)
    nc.compile()
    return nc


def get_nc():
    if "nc" not in _CACHE:
        _CACHE["nc"] = _build()
    return _CACHE["nc"]


def kernel(x, mask, wq_w, wq_b, wk_w, wk_b, wv_w, wv_b, dense_w, dense_b,
           _trace=False):
    bf = ml_dtypes.bfloat16
    x = np.asarray(x, dtype=np.float32)
    wq_w = np.asarray(wq_w, np.float32); wq_b = np.asarray(wq_b, np.float32)
    wk_w = np.asarray(wk_w, np.float32); wk_b = np.asarray(wk_b, np.float32)
    wv_w = np.asarray(wv_w, np.float32); wv_b = np.asarray(wv_b, np.float32)
    dense_w = np.asarray(dense_w, np.float32)
    dense_b = np.asarray(dense_b, np.float32)

    # causal masks, bf16: cols 0:128 = all -1e9; cols 128:256 = triangle
    # T[k, qq] = -1e9 where qq < k
    band = np.zeros((128, 256), np.float32)
    band[:, 0:128] = NEG
    k_idx = np.arange(128)[:, None]
    q_idx = np.arange(128)[None, :]
    band[:, 128:256] = np.where(q_idx < k_idx, NEG, 0.0)
    ident = np.eye(128, dtype=np.float32)
    onb = np.zeros((128, 128), np.float32)
    onb[64, 0:64] = 1.0   # broadcasts rowsum_A recip (at partition 64) to rows 0:64
    onb[0, 64:128] = 1.0  # broadcasts rowsum_B recip (at partition 0) to rows 64:128

    in_maps = []
    for core in range(N_CORES):
        b, hh = divmod(core, 2)
        sl = slice(DC * hh, DC * (hh + 1))
        in_maps.append({
            "xt": np.ascontiguousarray(x[b].T).astype(bf),
            "wq": np.ascontiguousarray(wq_w[:, sl]).astype(bf),
            "wk": np.ascontiguousarray(wk_w[:, sl]).astype(bf),
            "wv": np.ascontiguousarray(wv_w[:, sl]).astype(bf),
            "qb": np.ascontiguousarray(wq_b[sl]),
            "kb": np.ascontiguousarray(wk_b[sl]),
            "wd": np.ascontiguousarray(dense_w[sl, :]).astype(bf),
            "band": band.astype(bf), "idm": ident.astype(bf),
            "onb": onb.astype(bf),
        })
    nc = get_nc()
    res = run_bass_kernel_spmd(nc, in_maps, core_ids=list(range(N_CORES)),
                               trace=_trace)
    const = dense_b + wv_b @ dense_w  # bias terms deferred to host
    outs = np.empty((B, S, D), np.float32)
    for b in range(B):
        outs[b] = res.results[2 * b]["out"] + res.results[2 * b + 1]["out"] + const
    if _trace:
        kernel.last_result = res
    return outs
